# revision 18
# baseline (speedup 1.0000x reference)
"""Trainium2 SPMD kernel for DistanceContrastiveLoss.

Math:
  d2[i,j] = ||c_i||^2 + ||s_j||^2 - 2 c_i.s_j
  sim     = -exp(t) * sqrt(d2)
  loss    = 0.5*(CE(sim, diag) + CE(sim.T, diag))
          = 0.5*( mean_i(LSE_row_i - sim_ii) + mean_j(LSE_col_j - sim_jj) )

Sharding: rows of the 8192x8192 logits matrix are split across 8 cores
(1024 rows each). Each core computes, for its block:
  - row sums of exp(sim + B)
  - partial column sums of exp(sim + B)
The final log/diagonal/mean is a tiny O(N) host epilogue; partial column
sums from the 8 cores are summed on host (cheaper than a collective).

Device pipeline, per column-group phase (2048 cols) of 8 row-tiles:
  sub-phase A (sqrt ACT table):
    PE  : d2_partial = (-2c).s via fp8e4m3 DoubleRow matmuls (K=256 as
          2 k-chunks, one instruction per 512-col psum chunk) + s2_j via
          a plain bf16 K=128 rider (ones128 x s2bcast, s2c/64 split
          hi/lo across the K rows; small-K matmuls are slow on HW)
    ACT : v = sqrt(d2_partial + (c2_i + mean_s2)) -> bf16
  sub-phase B (exp ACT table):
    ACT : w = exp(-a*v + B) -> fp8e5m2 (+ fp32 row-sum accumulator)
    PE  : colsum strips: ones32.T @ w per 512-chunk, packed 4 strips per
          PSUM bank (tile_position), accumulated across the 8 row-tiles
Host quantization error (fp8 dot + bf16 s2 split + fp8e5m2 colsum +
bf16 v) is ~1.5e-4 relative on the final loss; tolerance is 2e-2.
"""

import contextlib
import os

os.environ.setdefault("MYCRO_LOCAL_CACHE", "1")

import numpy as np
import ml_dtypes

import concourse.bacc as bacc
import concourse.bass as bass
import concourse.mybir as mybir
import concourse.tile as tile
from concourse.bass_utils import run_bass_kernel_spmd

F32 = mybir.dt.float32
F16 = mybir.dt.float16
F8 = mybir.dt.float8e4
BF16 = mybir.dt.bfloat16
AF = mybir.ActivationFunctionType
DR = mybir.MatmulPerfMode.DoubleRow

# Full-problem geometry (hardcoded per the task contract).
N = 8192
D = 256
NCORES = 8
ROWS_PER_CORE = N // NCORES  # 1024
P = 128  # partitions


def build(n_rt: int, n_groups: int, debug: bool = False, reps: int = 1,
          hw_loop: bool = False, exp_wide: bool = False,
          sqrt2048: bool = False, noquad: bool = True,
          w8e5: bool = True, dverow: bool = False, gpp: int = 1,
          vbf16: bool = True):
    """Build the SPMD Bass program.

    n_rt      : number of 128-row tiles per core        (full: 8)
    n_groups  : number of 2048-wide column groups       (full: 4)
    hw_loop   : repeat the rep-body via a hardware For_i loop
    exp_wide  : exp over [128,4096] pairs + DVE row-sum reduces
                (vs per-rt [128,2048] exp with ACT accumulator)
    sqrt2048  : [128,2048] psum tiles (1 sqrt instr per rt); colsum
                strips then go through a transient psum tile + DVE add
                (vs persistent per-group colacc banks)
    """
    rows = P * n_rt
    cols = 2048 * n_groups
    n_ct = cols // 512  # 512-wide column tiles

    nc = bacc.Bacc(
        "TRN2", target_bir_lowering=False, debug=debug, num_devices=NCORES
    )

    lhs8_d = nc.dram_tensor("lhs8", [P, 2 * rows], F8, kind="ExternalInput")
    rhs8_d = nc.dram_tensor("rhs8", [P, 2 * cols], F8, kind="ExternalInput")
    s2k_d = nc.dram_tensor("s2k", [P, cols], BF16, kind="ExternalInput")
    c2_d = nc.dram_tensor("c2m", [P, n_rt], F32, kind="ExternalInput")
    cst_d = nc.dram_tensor("cst", [P, 2], F32, kind="ExternalInput")

    rowsums_d = nc.dram_tensor(
        "rowsums", [P, n_rt * n_groups], F32, kind="ExternalOutput"
    )
    colsums_d = nc.dram_tensor("colsums", [n_ct, 512], F32, kind="ExternalOutput")

    ctx = contextlib.ExitStack()
    with tile.TileContext(nc) as tc, ctx:
        inp = ctx.enter_context(tc.tile_pool(name="inp", bufs=1))
        cstp = ctx.enter_context(tc.tile_pool(name="cstp", bufs=1))
        outp = ctx.enter_context(tc.tile_pool(name="outp", bufs=1))
        if not exp_wide:
            vvp = ctx.enter_context(tc.tile_pool(name="vvp", bufs=n_rt * gpp + 1))
        wwp = ctx.enter_context(
            tc.tile_pool(name="wwp", bufs=(3 if exp_wide else 6))
        )
        wqp = ctx.enter_context(tc.tile_pool(name="wqp", bufs=4))
        d2p = ctx.enter_context(
            tc.tile_pool(name="d2p", bufs=2, space=bass.MemorySpace.PSUM)
        )
        if not sqrt2048:
            csp = ctx.enter_context(
                tc.tile_pool(name="csp", bufs=1, space=bass.MemorySpace.PSUM)
            )

        # ---- load inputs (small tensors first: they gate step 0) -------
        cst_sb = inp.tile([P, 2], F32, tag="cst")
        nc.sync.dma_start(out=cst_sb[:], in_=cst_d.ap()[:])
        c2_sb = inp.tile([P, n_rt], F32, tag="c2")
        nc.sync.dma_start(out=c2_sb[:], in_=c2_d.ap()[:])
        # chunk all big loads in consumption order: the first phase needs
        # only lhs8 row-tile 0, rhs8/s2k column chunk 0 -- emit those first
        # so the pipeline fills after ~1MB of DMA instead of ~4.5MB.
        lhs8 = inp.tile([P, 2, rows], F8, tag="lhs8")
        rhs8 = inp.tile([P, 2, cols], F8, tag="rhs8")
        s2k = inp.tile([P, cols], BF16, tag="s2k")
        for h in range(2):
            nc.sync.dma_start(
                out=lhs8[:, h, 0:P], in_=lhs8_d.ap()[:, h * rows : h * rows + P]
            )
        for cb in range(0, cols, 2048):
            # first chunk split finer: step 0 touches only cols 0..1023
            subs = (
                [(cb, 1024), (cb + 1024, 1024)] if cb == 0 else [(cb, 2048)]
            )
            for sb, sw in subs:
                for h in range(2):
                    nc.sync.dma_start(
                        out=rhs8[:, h, sb : sb + sw],
                        in_=rhs8_d.ap()[:, h * cols + sb : h * cols + sb + sw],
                    )
                nc.sync.dma_start(
                    out=s2k[:, sb : sb + sw], in_=s2k_d.ap()[:, sb : sb + sw]
                )
            if cb == 0:
                for h in range(2):
                    nc.sync.dma_start(
                        out=lhs8[:, h, P:rows],
                        in_=lhs8_d.ap()[:, h * rows + P : (h + 1) * rows],
                    )

        ones128 = cstp.tile([P, P], BF16)  # s2 rider weights (K=128)
        nc.vector.memset(ones128[:], 1.0)
        wdt = mybir.dt.float8e5 if w8e5 else BF16
        ones32 = cstp.tile([P, 32], wdt)  # column-sum stationary operand
        nc.vector.memset(ones32[:], 1.0)
        zeros128 = cstp.tile([P, P], BF16)  # zero weights: PSUM bank clear
        nc.vector.memset(zeros128[:], 0.0)
        zdum = cstp.tile([P, 512], BF16)  # zero-matmul moving operand
        nc.vector.memset(zdum[:], 0.0)

        rowsum = outp.tile([P, n_rt * n_groups], F32)
        cs_sb = outp.tile([P, 512 * n_groups], F32)
        if exp_wide:
            vbuf = outp.tile([P, n_rt * 2048], F16)
        if not sqrt2048:
            colacc = csp.tile([P, 512 * n_groups], F32)  # bank g per group

        vdt = BF16 if vbf16 else F16
        neg_a = cst_sb[:, 0:1]
        bias_b = cst_sb[:, 1:2]

        def emit_matmuls_1024(rt, g):
            lh = lhs8[:, :, rt * P : (rt + 1) * P]
            d2s = [d2p.tile([P, 1024], F32, name="d2") for _ in range(2)]
            for h in range(2):
                for jj in range(2):
                    colb = 2048 * g + 1024 * h + 512 * jj
                    nc.tensor.matmul(
                        d2s[h][:, 512 * jj : 512 * jj + 512],
                        lh,
                        rhs8[:, :, colb : colb + 512],
                        start=True, stop=False, perf_mode=DR,
                    )
            for h in range(2):
                for jj in range(2):
                    colb = 2048 * g + 1024 * h + 512 * jj
                    nc.tensor.matmul(
                        d2s[h][:, 512 * jj : 512 * jj + 512],
                        ones128[:],
                        s2k[:, colb : colb + 512],
                        start=False, stop=True,
                    )
            return d2s

        def emit_matmuls_2048(rt, g):
            lh = lhs8[:, :, rt * P : (rt + 1) * P]
            d2 = d2p.tile([P, 2048], F32, name="d2")
            for jj in range(4):
                colb = 2048 * g + 512 * jj
                nc.tensor.matmul(
                    d2[:, 512 * jj : 512 * jj + 512],
                    lh,
                    rhs8[:, :, colb : colb + 512],
                    start=True, stop=False, perf_mode=DR,
                )
            for jj in range(4):
                colb = 2048 * g + 512 * jj
                nc.tensor.matmul(
                    d2[:, 512 * jj : 512 * jj + 512],
                    ones128[:],
                    s2k[:, colb : colb + 512],
                    start=False, stop=True,
                )
            return d2

        def emit_strips(q, g, quad_idx, final_rep):
            """Column partial sums for one quad tile q [P, 2048]."""
            if sqrt2048:
                t = d2p.tile([P, 2048], F32, name="d2")  # transient slot
                nc.tensor.matmul(
                    t[:, 0:512], zeros128[:], zdum[:], start=True, stop=False
                )
                for k in range(4):
                    nc.tensor.matmul(
                        t[32 * k : 32 * k + 32, 0:512],
                        ones32[:],
                        q[:, 512 * k : 512 * k + 512],
                        start=False, stop=False,
                        tile_position=(0, 32 * k),
                        skip_group_check=True,
                    )
                nc.tensor.matmul(
                    t[:, 0:512], zeros128[:], zdum[:], start=False, stop=True
                )
                if quad_idx == 0:
                    nc.vector.tensor_copy(
                        cs_sb[:, 512 * g : 512 * g + 512], t[:, 0:512]
                    )
                else:
                    nc.vector.tensor_tensor(
                        cs_sb[:, 512 * g : 512 * g + 512],
                        cs_sb[:, 512 * g : 512 * g + 512],
                        t[:, 0:512],
                        op=mybir.AluOpType.add,
                    )
                    if final_rep:
                        for k in range(4):
                            nc.sync.dma_start(
                                out=colsums_d.ap()[4 * g + k : 4 * g + k + 1, :],
                                in_=cs_sb[
                                    32 * k : 32 * k + 1, 512 * g : 512 * g + 512
                                ],
                            )
            else:
                if quad_idx == 0:
                    nc.tensor.matmul(
                        colacc[:, 512 * g : 512 * g + 512],
                        zeros128[:], zdum[:], start=True, stop=False,
                    )
                for k in range(4):
                    nc.tensor.matmul(
                        colacc[32 * k : 32 * k + 32, 512 * g : 512 * g + 512],
                        ones32[:],
                        q[:, 512 * k : 512 * k + 512],
                        start=False, stop=False,
                        tile_position=(0, 32 * k),
                        skip_group_check=True,
                    )
                if quad_idx == 1:
                    nc.tensor.matmul(
                        colacc[:, 512 * g : 512 * g + 512],
                        zeros128[:], zdum[:], start=False, stop=True,
                    )
                    if final_rep:
                        nc.vector.tensor_copy(
                            cs_sb[:, 512 * g : 512 * g + 512],
                            colacc[:, 512 * g : 512 * g + 512],
                        )
                        for k in range(4):
                            nc.sync.dma_start(
                                out=colsums_d.ap()[4 * g + k : 4 * g + k + 1, :],
                                in_=cs_sb[
                                    32 * k : 32 * k + 1, 512 * g : 512 * g + 512
                                ],
                            )

        def rep_body(final_rep):
            assert not (exp_wide and gpp > 1)
            for gp in range(n_groups // gpp):
                glist = list(range(gp * gpp, (gp + 1) * gpp))
                # ---- sub-phase A: matmuls + sqrt ------------------------
                v_map = {}
                for g in glist:
                    for rt in range(n_rt):
                        if exp_wide:
                            v = vbuf[:, rt * 2048 : (rt + 1) * 2048]
                        else:
                            v = vvp.tile([P, 2048], vdt, name="v")
                        if sqrt2048:
                            d2 = emit_matmuls_2048(rt, g)
                            nc.scalar.activation(
                                v[:], d2[:], AF.Sqrt,
                                bias=c2_sb[:, rt : rt + 1], scale=1.0,
                            )
                        else:
                            d2s = emit_matmuls_1024(rt, g)
                            for h in range(2):
                                nc.scalar.activation(
                                    v[:, 1024 * h : 1024 * h + 1024],
                                    d2s[h][:], AF.Sqrt,
                                    bias=c2_sb[:, rt : rt + 1], scale=1.0,
                                )
                        v_map[(g, rt)] = v
                # ---- sub-phase B: exp + row sums + column sums ----------
                for g in glist:
                    if exp_wide:
                        w_pairs = []
                        for pr in range(n_rt // 2):
                            w = wwp.tile([P, 4096], BF16, name="w")
                            nc.scalar.activation(
                                w[:], vbuf[:, pr * 4096 : (pr + 1) * 4096],
                                AF.Exp, bias=bias_b, scale=neg_a,
                            )
                            for half in range(2):
                                rt = 2 * pr + half
                                nc.vector.tensor_reduce(
                                    rowsum[:, rt * n_groups + g : rt * n_groups + g + 1],
                                    w[:, half * 2048 : half * 2048 + 2048],
                                    axis=mybir.AxisListType.XYZW,
                                    op=mybir.AluOpType.add,
                                )
                            w_pairs.append(w)
                            if pr % 2 != 1:
                                continue
                            wa, wb_ = w_pairs[-2:]
                            p0 = wqp.tile([P, 2048], BF16, name="wq")
                            nc.vector.tensor_tensor(
                                p0[:], wa[:, 0:2048], wa[:, 2048:4096],
                                op=mybir.AluOpType.add,
                            )
                            p1 = wqp.tile([P, 2048], BF16, name="wq")
                            nc.vector.tensor_tensor(
                                p1[:], wb_[:, 0:2048], wb_[:, 2048:4096],
                                op=mybir.AluOpType.add,
                            )
                            q = wqp.tile([P, 2048], BF16, name="wq")
                            nc.vector.tensor_tensor(
                                q[:], p0[:], p1[:], op=mybir.AluOpType.add
                            )
                            emit_strips(q, g, pr // 2, final_rep)
                        continue
                    w_tiles = []
                    for rt in range(n_rt):
                        w = wwp.tile([P, 2048], wdt, name="w")
                        if dverow:
                            nc.scalar.activation(
                                w[:], v_map[(g, rt)][:], AF.Exp,
                                bias=bias_b, scale=neg_a,
                            )
                            nc.vector.tensor_reduce(
                                rowsum[
                                    :, rt * n_groups + g : rt * n_groups + g + 1
                                ],
                                w[:],
                                axis=mybir.AxisListType.XYZW,
                                op=mybir.AluOpType.add,
                            )
                        else:
                            nc.scalar.activation(
                                w[:], v_map[(g, rt)][:], AF.Exp,
                                bias=bias_b, scale=neg_a,
                                accum_out=rowsum[
                                    :, rt * n_groups + g : rt * n_groups + g + 1
                                ],
                            )
                        if noquad:
                            if rt == 0:
                                nc.tensor.matmul(
                                    colacc[:, 512 * g : 512 * g + 512],
                                    zeros128[:], zdum[:], start=True, stop=False,
                                )
                            for k in range(4):
                                nc.tensor.matmul(
                                    colacc[
                                        32 * k : 32 * k + 32,
                                        512 * g : 512 * g + 512,
                                    ],
                                    ones32[:],
                                    w[:, 512 * k : 512 * k + 512],
                                    start=False, stop=False,
                                    tile_position=(0, 32 * k),
                                    skip_group_check=True,
                                )
                            if rt == n_rt - 1:
                                nc.tensor.matmul(
                                    colacc[:, 512 * g : 512 * g + 512],
                                    zeros128[:], zdum[:], start=False, stop=True,
                                )
                                if final_rep:
                                    nc.vector.tensor_copy(
                                        cs_sb[:, 512 * g : 512 * g + 512],
                                        colacc[:, 512 * g : 512 * g + 512],
                                    )
                                    for k in range(4):
                                        nc.sync.dma_start(
                                            out=colsums_d.ap()[
                                                4 * g + k : 4 * g + k + 1, :
                                            ],
                                            in_=cs_sb[
                                                32 * k : 32 * k + 1,
                                                512 * g : 512 * g + 512,
                                            ],
                                        )
                            continue
                        w_tiles.append(w)
                        if rt % 4 != 3:
                            continue
                        w0, w1, w2, w3 = w_tiles[-4:]
                        p0 = wqp.tile([P, 2048], BF16, name="wq")
                        nc.vector.tensor_tensor(
                            p0[:], w0[:], w1[:], op=mybir.AluOpType.add
                        )
                        p1 = wqp.tile([P, 2048], BF16, name="wq")
                        nc.vector.tensor_tensor(
                            p1[:], w2[:], w3[:], op=mybir.AluOpType.add
                        )
                        q = wqp.tile([P, 2048], BF16, name="wq")
                        nc.vector.tensor_tensor(
                            q[:], p0[:], p1[:], op=mybir.AluOpType.add
                        )
                        emit_strips(q, g, rt // 4, final_rep)

        if hw_loop:
            with tc.For_i(0, reps, 1):
                rep_body(False)
            for g in range(n_groups):
                if not sqrt2048:
                    nc.vector.tensor_copy(
                        cs_sb[:, 512 * g : 512 * g + 512],
                        colacc[:, 512 * g : 512 * g + 512],
                    )
                for k in range(4):
                    nc.sync.dma_start(
                        out=colsums_d.ap()[4 * g + k : 4 * g + k + 1, :],
                        in_=cs_sb[32 * k : 32 * k + 1, 512 * g : 512 * g + 512],
                    )
        else:
            for _rep in range(reps):
                rep_body(_rep == reps - 1)

        nc.sync.dma_start(out=rowsums_d.ap()[:], in_=rowsum[:])

    nc.compile()
    return nc


def host_prep(cond_feature, sol_feature, temperature, n_rt=8, n_groups=4):
    """Build per-core input maps + host-side scalars."""
    c = np.asarray(cond_feature, dtype=np.float32).reshape(-1, D)
    s = np.asarray(sol_feature, dtype=np.float32).reshape(-1, D)
    n = c.shape[0]
    rows = P * n_rt
    cols = 2048 * n_groups

    a = float(np.exp(np.float64(np.asarray(temperature))))
    c2 = np.sum(c.astype(np.float64) ** 2, axis=1)
    s2 = np.sum(s.astype(np.float64) ** 2, axis=1)
    ms2 = float(np.mean(s2))
    d2_mean = float(np.mean(c2) + ms2)
    B = a * float(np.sqrt(max(d2_mean, 1e-6)))

    q8 = lambda x: np.asarray(x, np.float32).astype(ml_dtypes.float8_e4m3)
    cq = q8(-2.0 * c)  # [n, D]
    sq = q8(s)[:cols]  # [cols, D]
    s2c = s2[:cols] - ms2
    # rider rows: 0..63 carry hi=bf16(s2c/64), 64..127 carry the residual/64
    s2hi = np.asarray(s2c / 64.0, np.float32).astype(ml_dtypes.bfloat16)
    s2res = s2c - 64.0 * s2hi.astype(np.float64)
    s2lo = np.asarray(s2res / 64.0, np.float32).astype(ml_dtypes.bfloat16)
    s2k = np.empty((P, cols), ml_dtypes.bfloat16)
    s2k[:64, :] = s2hi[None, :]
    s2k[64:, :] = s2lo[None, :]

    # [K, 2, cols] k-chunk-major layouts
    rhs8 = np.ascontiguousarray(
        np.stack([sq[:, :P].T, sq[:, P:].T], axis=1).reshape(P, 2 * cols)
    )
    cst = np.empty((P, 2), dtype=np.float32)
    cst[:, 0] = -a
    cst[:, 1] = B

    in_maps = []
    ncores = max(1, n // rows)
    for k in range(ncores):
        cq_k = cq[k * rows : (k + 1) * rows]  # [rows, D]
        lhs8_k = np.ascontiguousarray(
            np.stack([cq_k[:, :P].T, cq_k[:, P:].T], axis=1).reshape(P, 2 * rows)
        )
        c2_k = (
            (c2[k * rows : (k + 1) * rows] + ms2)
            .astype(np.float32)
            .reshape(n_rt, P)
            .T.copy()
        )
        in_maps.append(
            {
                "lhs8": lhs8_k.view(np.uint8),
                "rhs8": rhs8.view(np.uint8),
                "s2k": s2k.view(np.uint16),
                "c2m": c2_k,
                "cst": cst,
            }
        )

    # diagonal of sim in float64 (tiny O(N*D) host cost)
    dd = np.sqrt(np.maximum(np.sum((c.astype(np.float64) - s.astype(np.float64)) ** 2, axis=1), 0.0))
    sim_diag = -a * dd
    return in_maps, a, B, sim_diag


def host_post(results, B, sim_diag, n_rt=8, n_groups=4):
    """Combine per-core rowsums/colsums into the scalar loss."""
    lse_rows = []
    col_total = None
    for res in results:
        rs = np.asarray(res["rowsums"], dtype=np.float64)  # [P, n_rt*n_groups]
        # per-row total = sum over groups; row order within core: rt*128 + p
        rt_tot = rs.reshape(P, n_rt, n_groups).sum(axis=2)  # [P, n_rt]
        lse_rows.append(np.log(rt_tot.T.reshape(-1)) - B)  # [rows]
        cs = np.asarray(res["colsums"], dtype=np.float64).reshape(-1)
        col_total = cs if col_total is None else col_total + cs
    lse_row = np.concatenate(lse_rows)
    lse_col = np.log(col_total) - B

    loss_row = np.mean(lse_row - sim_diag[: lse_row.shape[0]])
    loss_col = np.mean(lse_col - sim_diag[: lse_col.shape[0]])
    return np.float32(0.5 * (loss_row + loss_col))


_NC_CACHE = {}


def _get_nc(n_rt=8, n_groups=4):
    key = (n_rt, n_groups)
    if key not in _NC_CACHE:
        _NC_CACHE[key] = build(n_rt, n_groups)
    return _NC_CACHE[key]


def run(cond_feature, sol_feature, temperature, trace=False):
    nc = _get_nc()
    in_maps, a, B, sim_diag = host_prep(cond_feature, sol_feature, temperature)
    res = run_bass_kernel_spmd(
        nc, in_maps, core_ids=list(range(NCORES)), trace=trace
    )
    loss = host_post(res.results, B, sim_diag)
    return loss, res


def kernel(cond_feature, sol_feature, temperature):
    loss, _ = run(cond_feature, sol_feature, temperature, trace=False)
    return loss
